# revision 33
# baseline (speedup 1.0000x reference)
"""Causal self-attention (B=4, S=2048, D=1024, H=16, Hd=64) on 8 trn2 cores.

Sharding: core = (batch b in 0..3) x (head-group hg in 0..1, 8 heads each).
Each core computes QKV projection for its batch restricted to its 8 heads
(tensor-parallel column slice of qkv_w), full causal attention for those
heads, and a partial output projection (row slice of proj_w). Host sums the
two head-group partials per batch and adds proj_b.

v2 structure (all bf16 matmuls, f32 PSUM accumulation):
  - x is transposed on the host; xT [D,S] is DMA'd straight into per-(dtile,
    chunk) SBUF tiles (no PE transpose phase).
  - Per (pair of heads, 512-query chunk): qk projection -> scores (two K=64
    matmuls at row groups 0/64 run concurrently on the PE) -> exp on ACT ->
    causal triangle mask on DVE -> AV pair (M=64 col-tiled: head0 -> psum
    rows 0:64, head1 -> rows 64:128, concurrent) + ones-matmul pair for the
    softmax denominators (psum rows 0 and 32 of a [33,512] tile, concurrent).
  - Normalize via DVE reciprocal + gpsimd partition_broadcast + DVE mul.
  - Output projection is emitted per chunk right after the last pair's
    attention for that chunk, so it fills PE idle slots while ACT works.
  - Program order interleaves v-projection and later pairs' qk projection
    between attention chunks; exp on ACT is the critical resource and the
    PE work drafts behind it.
"""
import sys

for _p in ("/opt/trn_rl_repo", "/root/.axon_site/_ro/trn_rl_repo"):
    if _p not in sys.path:
        sys.path.append(_p)

import ml_dtypes
import numpy as np

import concourse.bass as bass
import concourse.tile as tile
from concourse import bacc, mybir
from concourse.bass_utils import run_bass_kernel_spmd
from concourse.masks import make_upper_triangular

f32 = mybir.dt.float32
bf16 = mybir.dt.bfloat16
Exp = mybir.ActivationFunctionType.Exp

B, S, D = 4, 2048, 1024
H, HD = 16, 64
F = 512            # features per core (8 heads)
NHL = 8            # heads per core
NDT = D // 128     # 8 d-tiles
NST = S // 128     # 16 s-tiles
NCH = S // 512     # 4 query chunks
N_CORES = 8
SCALE = 1.0 / 8.0  # 1/sqrt(HD)


def build_program():
    nc = bacc.Bacc("TRN2", target_bir_lowering=False, debug=False,
                   num_devices=N_CORES)
    xT_d = nc.dram_tensor("xT", [D, S], bf16, kind="ExternalInput").ap()
    wq_d = nc.dram_tensor("wq", [D, F], bf16, kind="ExternalInput").ap()
    wk_d = nc.dram_tensor("wk", [D, F], bf16, kind="ExternalInput").ap()
    wv_d = nc.dram_tensor("wv", [D, F], bf16, kind="ExternalInput").ap()
    bq_d = nc.dram_tensor("bq", [F], f32, kind="ExternalInput").ap()
    bk_d = nc.dram_tensor("bk", [F], f32, kind="ExternalInput").ap()
    bv_d = nc.dram_tensor("bv", [F], f32, kind="ExternalInput").ap()
    wp_d = nc.dram_tensor("wp", [F, D], bf16, kind="ExternalInput").ap()
    out_d = nc.dram_tensor("out", [S, D], f32, kind="ExternalOutput").ap()

    with tile.TileContext(nc) as tc:
        build_body(nc, tc, xT_d, wq_d, wk_d, wv_d, bq_d, bk_d, bv_d, wp_d,
                   out_d)
    nc.compile()
    return nc


def build_body(nc, tc, xT_d, wq_d, wk_d, wv_d, bq_d, bk_d, bv_d, wp_d, out_d):
    consts = tc.alloc_tile_pool(name="consts", bufs=1)
    wpool = tc.alloc_tile_pool(name="wpool", bufs=1)
    bpool = tc.alloc_tile_pool(name="bpool", bufs=1)
    xpool = tc.alloc_tile_pool(name="xpool", bufs=1)
    qkpool = tc.alloc_tile_pool(name="qkpool", bufs=1)
    vpool = tc.alloc_tile_pool(name="vpool", bufs=1)
    aopool = tc.alloc_tile_pool(name="aopool", bufs=1)
    ppool = tc.alloc_tile_pool(name="ppool", bufs=6)
    npool = tc.alloc_tile_pool(name="npool", bufs=4)
    ostage = tc.alloc_tile_pool(name="ostage", bufs=3)
    # PSUM: ps_pr(1 bank x2) + ps_sc(2 banks x2) + ps_av(1 bank x2) = 8
    ps_pr = tc.alloc_tile_pool(name="ps_pr", bufs=2, space="PSUM")
    ps_sc = tc.alloc_tile_pool(name="ps_sc", bufs=2, space="PSUM")
    ps_av = tc.alloc_tile_pool(name="ps_av", bufs=2, space="PSUM")

    tri = consts.tile([128, 128], bf16, tag="tri")    # 1 on/above diag
    make_upper_triangular(nc, tri[:], val=1.0, diag=True)

    # ---------------- weight / bias / xT DMAs -----------------
    # Inputs ride three DMA rings in parallel. The scalar queue carries only
    # the wq tiles, all issued before any exp is enqueued, so ACT is never
    # head-blocked. Order: wq (scalar) || xT(c0)+wk (sync) || wv (gpsimd),
    # so the q projection can start ~8us in.
    wq_sb, wk_sb = [], []
    for dt_ in range(NDT):
        w1 = wpool.tile([128, F], bf16, tag=f"wq{dt_}", name=f"wq{dt_}")
        nc.scalar.dma_start(w1[:], wq_d[dt_ * 128:(dt_ + 1) * 128, :])
        wq_sb.append(w1)

    xT = [[xpool.tile([128, 512], bf16, tag=f"xT{d}_{c}", name=f"xT{d}_{c}")
           for c in range(NCH)] for d in range(NDT)]
    for dt_ in range(NDT):
        nc.sync.dma_start(xT[dt_][0][:],
                          xT_d[dt_ * 128:(dt_ + 1) * 128, 0:512])

    for dt_ in range(NDT):
        w2 = wpool.tile([128, F], bf16, tag=f"wk{dt_}", name=f"wk{dt_}")
        nc.sync.dma_start(w2[:], wk_d[dt_ * 128:(dt_ + 1) * 128, :])
        wk_sb.append(w2)

    wtv = []
    for dt_ in range(NDT):
        w = wpool.tile([128, F], bf16, tag=f"wv{dt_}", name=f"wv{dt_}")
        nc.gpsimd.dma_start(w[:], wv_d[dt_ * 128:(dt_ + 1) * 128, :])
        wtv.append(w)

    for c in range(1, NCH):
        for dt_ in range(NDT):
            nc.sync.dma_start(
                xT[dt_][c][:],
                xT_d[dt_ * 128:(dt_ + 1) * 128, c * 512:(c + 1) * 512])

    bqcol, bkcol = [], []
    for pair in range(4):
        bc_ = bpool.tile([128, 1], f32, tag=f"bq{pair}", name=f"bq{pair}")
        nc.gpsimd.dma_start(bc_[:], bq_d[pair * 128:(pair + 1) * 128, None])
        bqcol.append(bc_)
        bc2 = bpool.tile([128, 1], f32, tag=f"bk{pair}", name=f"bk{pair}")
        nc.gpsimd.dma_start(bc2[:], bk_d[pair * 128:(pair + 1) * 128, None])
        bkcol.append(bc2)
    bvrow = bpool.tile([1, F], f32, tag="bvrow")
    nc.gpsimd.dma_start(bvrow[:], bv_d[None, :])
    bvb = bpool.tile([128, F], f32, tag="bvb")
    nc.gpsimd.partition_broadcast(bvb[:], bvrow[:])

    wp = []
    for ft in range(4):
        w = wpool.tile([128, D], bf16, tag=f"wp{ft}", name=f"wp{ft}")
        nc.gpsimd.dma_start(w[:], wp_d[ft * 128:(ft + 1) * 128, :])
        wp.append(w)

    # ---------------- persistent SBUF arrays -----------------
    qT = [[qkpool.tile([128, 512], bf16, tag=f"qT{p}_{c}", name=f"qT{p}_{c}")
           for c in range(NCH)] for p in range(4)]
    kT = [[qkpool.tile([128, 512], bf16, tag=f"kT{p}_{c}", name=f"kT{p}_{c}")
           for c in range(NCH)] for p in range(4)]
    # v tiles: [128 keys, 8 heads, 64+1] -- col 64 of each head is 1.0 so the
    # AV matmul row 64 accumulates the softmax denominator.
    v_sb = [vpool.tile([128, NHL, HD + 1], bf16, tag=f"v{st}", name=f"v{st}")
            for st in range(NST)]
    aoT = [[aopool.tile([128, 512], bf16, tag=f"ao{p}_{c}", name=f"ao{p}_{c}")
            for c in range(NCH)] for p in range(4)]

    def qk_half(pair, cs, wsb, bcol, dst):
        # both chunks stream through the same stationary weight tile per dt
        # so the PE can reuse the loaded weights (halves LDWEIGHTS traffic)
        pss = [ps_pr.tile([128, 512], f32, tag="pr", name=f"qk{pair}_{c}")
               for c in cs]
        for dt_ in range(NDT):
            for ps, c in zip(pss, cs):
                nc.tensor.matmul(
                    ps[:], wsb[dt_][:, pair * 128:(pair + 1) * 128],
                    xT[dt_][c][:],
                    start=(dt_ == 0), stop=(dt_ == NDT - 1))
            if dt_ < NDT - 1:
                yield
        for ps, c in zip(pss, cs):
            nc.vector.tensor_scalar_add(dst[pair][c][:], ps[:], bcol[pair][:])
        yield

    def gen_qk(pair, cs):
        yield from qk_half(pair, cs, wq_sb, bqcol, qT)
        yield from qk_half(pair, cs, wk_sb, bkcol, kT)

    def gen_v(st):
        ps = ps_pr.tile([128, 512], f32, tag="pr", name=f"v{st}")
        for dt_ in range(NDT):
            nc.tensor.matmul(
                ps[:], xT[dt_][st // 4][:, (st % 4) * 128:(st % 4 + 1) * 128],
                wtv[dt_][:],
                start=(dt_ == 0), stop=(dt_ == NDT - 1))
            if dt_ % 2 == 1 and dt_ < NDT - 1:
                yield
        nc.vector.tensor_add(
            v_sb[st][:, :, 0:HD],
            ps[:].rearrange("p (h d) -> p h d", h=NHL),
            bvb[:].rearrange("p (h d) -> p h d", h=NHL))
        nc.vector.memset(v_sb[st][:, :, HD:HD + 1], 1.0)
        yield

    def gen_proj(c, st):
        # both d-halves share each stationary aoT slice (LDWEIGHTS reuse)
        pos = [ps_pr.tile([128, 512], f32, tag="pr", name=f"po{st}_{n}")
               for n in range(2)]
        for p4 in range(4):
            for nch_ in range(2):
                nc.tensor.matmul(
                    pos[nch_][:],
                    aoT[p4][c][:, (st % 4) * 128:(st % 4 + 1) * 128],
                    wp[p4][:, nch_ * 512:(nch_ + 1) * 512],
                    start=(p4 == 0), stop=(p4 == 3))
            if p4 < 3:
                yield
        for nch_ in range(2):
            ot = ostage.tile([128, 512], f32, tag="ot", name=f"ot{st}_{nch_}")
            nc.vector.tensor_copy(ot[:], pos[nch_][:])
            nc.sync.dma_start(
                out_d[st * 128:(st + 1) * 128,
                      nch_ * 512:(nch_ + 1) * 512], ot[:])
        yield

    # ---- filler scheduling: the per-engine queues execute in program
    # order, so PE filler work (qkv / output projections) must be emitted
    # BETWEEN attention matmuls in ~2-matmul quanta — big lumps would delay
    # the next scores matmul and starve ACT.
    filler_q = []           # list of (tags_tuple, generator_fn)
    emitted = set()
    cur = [None, None]      # (tags_tuple, running generator)

    def pump(steps):
        for _ in range(steps):
            if cur[1] is None:
                if not filler_q:
                    return
                tags, genf = filler_q.pop(0)
                cur[0], cur[1] = tags, genf()
            try:
                next(cur[1])
            except StopIteration:
                emitted.update(cur[0])
                cur[1] = None

    def require(tags):
        for _ in range(10000):
            if all(t in emitted for t in tags):
                return
            assert filler_q or cur[1] is not None, f"missing {tags}"
            pump(1)
        raise AssertionError("require did not converge")

    def norm_chunk(pair, c, pav):
        # normalize: aoT[pair][c] = pav * (1/denom) broadcast on partitions
        for i in range(2):
            sums_sb = npool.tile([1, 512], f32, tag="sums",
                                 name=f"s{pair}_{c}_{i}")
            nc.vector.tensor_copy(sums_sb[:], pav[i][HD:HD + 1, :])
            r = npool.tile([1, 512], f32, tag="r", name=f"r{pair}_{c}_{i}")
            nc.vector.reciprocal_approx_fast(r[:], sums_sb[:])
            bc_ = npool.tile([HD, 512], f32, tag="bc",
                             name=f"bc{pair}_{c}_{i}")
            nc.gpsimd.partition_broadcast(bc_[:], r[:])
            nc.vector.tensor_mul(aoT[pair][c][64 * i:64 * i + 64, :],
                                 pav[i][0:HD, :], bc_[:])

    # ---------------- emission order -----------------
    for _ in gen_qk(0, (0,)):
        pass
    emitted.add("qk0_0")
    for st in range(0, 4):
        filler_q.append(((f"v{st}",), lambda st=st: gen_v(st)))
    filler_q.append((("qk0_1",), lambda: gen_qk(0, (1,))))
    for st in range(4, 8):
        filler_q.append(((f"v{st}",), lambda st=st: gen_v(st)))
    filler_q.append((("qk0_2", "qk0_3"), lambda: gen_qk(0, (2, 3))))
    for st in range(8, 16):
        filler_q.append(((f"v{st}",), lambda st=st: gen_v(st)))
    for p in range(1, 4):
        for cp in range(2):
            filler_q.append(((f"qk{p}_{2 * cp}", f"qk{p}_{2 * cp + 1}"),
                             lambda p=p, cp=cp: gen_qk(p, (2 * cp,
                                                          2 * cp + 1))))

    # Flat software-pipelined attention stream over all (pair, chunk, kt):
    # AV matmuls trail their exp by one global step, across chunk and pair
    # boundaries, so the PE queue never knots on an ACT-dependent matmul.
    steps = [(pair, c, kt) for pair in range(4) for c in range(NCH)
             for kt in range(4 * c + 4)]
    pav_cur = {}
    pending = None      # (pair, c, kt, pp, lo, is_last)

    def flush_pending():
        if pending is None:
            return
        pair, c, kt, pp, lo, is_last = pending
        require([f"v{kt}"])
        if kt == 0:
            pav_cur[(pair, c)] = [
                ps_av.tile([HD + 1, 512], f32, tag="av",
                           name=f"av{pair}_{c}_{i}") for i in range(2)]
        pav = pav_cur[(pair, c)]
        for i in range(2):
            nc.tensor.matmul(pav[i][:, lo:512],
                             v_sb[kt][:, 2 * pair + i, :],
                             pp[:, i, lo:512],
                             start=(kt == 0), stop=is_last)
        if is_last:
            norm_chunk(pair, c, pav)
            del pav_cur[(pair, c)]
            if pair == 3:
                for st in range(4 * c, 4 * c + 4):
                    filler_q.append(
                        ((f"po{st}",),
                         lambda c=c, st=st: gen_proj(c, st)))

    for pair, c, kt in steps:
        if kt == 0:
            require([f"qk{pair}_{c}"])
        j = kt - 4 * c          # >= 0 on diagonal k-tiles
        lo = 0 if j < 0 else 128 * j
        sc = ps_sc.tile([128, 2, 512], f32, tag="sc")
        for i in range(2):
            nc.tensor.matmul(
                sc[:, i, lo:512],
                kT[pair][kt // 4][64 * i:64 * i + 64,
                                  (kt % 4) * 128:(kt % 4 + 1) * 128],
                qT[pair][c][64 * i:64 * i + 64, lo:512],
                start=True, stop=True)
        pp = ppool.tile([128, 2, 512], bf16, tag="pp")
        nc.scalar.activation(pp[:, :, lo:512], sc[:, :, lo:512], Exp,
                             scale=SCALE)
        if j >= 0:
            nc.vector.tensor_mul(
                pp[:, :, lo:lo + 128], pp[:, :, lo:lo + 128],
                tri[:, None, :].broadcast_to([128, 2, 128]))
        pump(2)
        flush_pending()
        pending = (pair, c, kt, pp, lo, kt == 4 * c + 3)
    flush_pending()
    pending = None
    while filler_q or cur[1] is not None:
        pump(1)

    for pool in (ostage, npool, ppool, ps_av, ps_sc, ps_pr, aopool,
                 vpool, qkpool, xpool, bpool, wpool, consts):
        pool.release()


_NC_CACHE = None


def _get_program():
    global _NC_CACHE
    if _NC_CACHE is None:
        _NC_CACHE = build_program()
    return _NC_CACHE


def make_in_maps(x, qkv_w, qkv_b, proj_w):
    in_maps = []
    for cid in range(N_CORES):
        b, hg = cid // 2, cid % 2
        bf = ml_dtypes.bfloat16
        in_maps.append({
            "xT": np.ascontiguousarray(x[b].T).astype(bf),
            "wq": np.ascontiguousarray(qkv_w[:, hg * F:(hg + 1) * F]).astype(bf),
            "wk": np.ascontiguousarray(qkv_w[:, D + hg * F:D + (hg + 1) * F]).astype(bf),
            "wv": np.ascontiguousarray(qkv_w[:, 2 * D + hg * F:2 * D + (hg + 1) * F]).astype(bf),
            "bq": np.ascontiguousarray(qkv_b[hg * F:(hg + 1) * F]),
            "bk": np.ascontiguousarray(qkv_b[D + hg * F:D + (hg + 1) * F]),
            "bv": np.ascontiguousarray(qkv_b[2 * D + hg * F:2 * D + (hg + 1) * F]),
            "wp": np.ascontiguousarray(proj_w[hg * F:(hg + 1) * F, :]).astype(bf),
        })
    return in_maps


LAST_RESULTS = None


def kernel(x, qkv_w, qkv_b, proj_w, proj_b, _trace=False):
    global LAST_RESULTS
    nc = _get_program()
    in_maps = make_in_maps(np.asarray(x, dtype=np.float32),
                           np.asarray(qkv_w, dtype=np.float32),
                           np.asarray(qkv_b, dtype=np.float32),
                           np.asarray(proj_w, dtype=np.float32))
    res = run_bass_kernel_spmd(nc, in_maps, core_ids=list(range(N_CORES)),
                               trace=_trace)
    LAST_RESULTS = res
    out = np.empty((B, S, D), dtype=np.float32)
    for b in range(B):
        out[b] = res.results[2 * b]["out"] + res.results[2 * b + 1]["out"]
    out += proj_b.astype(np.float32)
    return out


# revision 34
# speedup vs baseline: 1.0435x; 1.0435x over previous
"""Causal self-attention (B=4, S=2048, D=1024, H=16, Hd=64) on 8 trn2 cores.

Sharding: core = (batch b in 0..3) x (head-group hg in 0..1, 8 heads each).
Each core computes QKV projection for its batch restricted to its 8 heads
(tensor-parallel column slice of qkv_w), full causal attention for those
heads, and a partial output projection (row slice of proj_w). Host sums the
two head-group partials per batch and adds proj_b.

v2 structure (all bf16 matmuls, f32 PSUM accumulation):
  - x is transposed on the host; xT [D,S] is DMA'd straight into per-(dtile,
    chunk) SBUF tiles (no PE transpose phase).
  - Per (pair of heads, 512-query chunk): qk projection -> scores (two K=64
    matmuls at row groups 0/64 run concurrently on the PE) -> exp on ACT ->
    causal triangle mask on DVE -> AV pair (M=64 col-tiled: head0 -> psum
    rows 0:64, head1 -> rows 64:128, concurrent) + ones-matmul pair for the
    softmax denominators (psum rows 0 and 32 of a [33,512] tile, concurrent).
  - Normalize via DVE reciprocal + gpsimd partition_broadcast + DVE mul.
  - Output projection is emitted per chunk right after the last pair's
    attention for that chunk, so it fills PE idle slots while ACT works.
  - Program order interleaves v-projection and later pairs' qk projection
    between attention chunks; exp on ACT is the critical resource and the
    PE work drafts behind it.
"""
import sys

for _p in ("/opt/trn_rl_repo", "/root/.axon_site/_ro/trn_rl_repo"):
    if _p not in sys.path:
        sys.path.append(_p)

import ml_dtypes
import numpy as np

import concourse.bass as bass
import concourse.tile as tile
from concourse import bacc, mybir
from concourse.bass_utils import run_bass_kernel_spmd
from concourse.masks import make_upper_triangular

f32 = mybir.dt.float32
bf16 = mybir.dt.bfloat16
Exp = mybir.ActivationFunctionType.Exp

B, S, D = 4, 2048, 1024
H, HD = 16, 64
F = 512            # features per core (8 heads)
NHL = 8            # heads per core
NDT = D // 128     # 8 d-tiles
NST = S // 128     # 16 s-tiles
NCH = S // 512     # 4 query chunks
N_CORES = 8
SCALE = 1.0 / 8.0  # 1/sqrt(HD)


def build_program():
    nc = bacc.Bacc("TRN2", target_bir_lowering=False, debug=False,
                   num_devices=N_CORES)
    xT_d = nc.dram_tensor("xT", [D, S], bf16, kind="ExternalInput").ap()
    wq_d = nc.dram_tensor("wq", [D, F], bf16, kind="ExternalInput").ap()
    wk_d = nc.dram_tensor("wk", [D, F], bf16, kind="ExternalInput").ap()
    wv_d = nc.dram_tensor("wv", [D, F], bf16, kind="ExternalInput").ap()
    bq_d = nc.dram_tensor("bq", [F], f32, kind="ExternalInput").ap()
    bk_d = nc.dram_tensor("bk", [F], f32, kind="ExternalInput").ap()
    bv_d = nc.dram_tensor("bv", [F], f32, kind="ExternalInput").ap()
    wp_d = nc.dram_tensor("wp", [F, D], bf16, kind="ExternalInput").ap()
    out_d = nc.dram_tensor("out", [S, D], f32, kind="ExternalOutput").ap()

    with tile.TileContext(nc) as tc:
        build_body(nc, tc, xT_d, wq_d, wk_d, wv_d, bq_d, bk_d, bv_d, wp_d,
                   out_d)
    nc.compile()
    return nc


def build_body(nc, tc, xT_d, wq_d, wk_d, wv_d, bq_d, bk_d, bv_d, wp_d, out_d):
    consts = tc.alloc_tile_pool(name="consts", bufs=1)
    wpool = tc.alloc_tile_pool(name="wpool", bufs=1)
    bpool = tc.alloc_tile_pool(name="bpool", bufs=1)
    xpool = tc.alloc_tile_pool(name="xpool", bufs=1)
    qkpool = tc.alloc_tile_pool(name="qkpool", bufs=1)
    vpool = tc.alloc_tile_pool(name="vpool", bufs=1)
    aopool = tc.alloc_tile_pool(name="aopool", bufs=1)
    ppool = tc.alloc_tile_pool(name="ppool", bufs=6)
    npool = tc.alloc_tile_pool(name="npool", bufs=4)
    ostage = tc.alloc_tile_pool(name="ostage", bufs=3)
    # PSUM: ps_pr(1 bank x2) + ps_sc(2 banks x2) + ps_av(1 bank x2) = 8
    ps_pr = tc.alloc_tile_pool(name="ps_pr", bufs=2, space="PSUM")
    ps_sc = tc.alloc_tile_pool(name="ps_sc", bufs=2, space="PSUM")
    ps_av = tc.alloc_tile_pool(name="ps_av", bufs=2, space="PSUM")

    tri = consts.tile([128, 128], bf16, tag="tri")    # 1 on/above diag
    make_upper_triangular(nc, tri[:], val=1.0, diag=True)

    # ---------------- weight / bias / xT DMAs -----------------
    # Inputs ride three DMA rings in parallel. The scalar queue carries only
    # the wq tiles, all issued before any exp is enqueued, so ACT is never
    # head-blocked. Order: wq (scalar) || xT(c0)+wk (sync) || wv (gpsimd),
    # so the q projection can start ~8us in.
    wq_sb, wk_sb = [], []
    for dt_ in range(NDT):
        w1 = wpool.tile([128, F], bf16, tag=f"wq{dt_}", name=f"wq{dt_}")
        nc.scalar.dma_start(w1[:], wq_d[dt_ * 128:(dt_ + 1) * 128, :])
        wq_sb.append(w1)

    xT = [[xpool.tile([128, 512], bf16, tag=f"xT{d}_{c}", name=f"xT{d}_{c}")
           for c in range(NCH)] for d in range(NDT)]
    for dt_ in range(NDT):
        nc.sync.dma_start(xT[dt_][0][:],
                          xT_d[dt_ * 128:(dt_ + 1) * 128, 0:512])

    for dt_ in range(NDT):
        w2 = wpool.tile([128, F], bf16, tag=f"wk{dt_}", name=f"wk{dt_}")
        nc.sync.dma_start(w2[:], wk_d[dt_ * 128:(dt_ + 1) * 128, :])
        wk_sb.append(w2)

    wtv = []
    for dt_ in range(NDT):
        w = wpool.tile([128, F], bf16, tag=f"wv{dt_}", name=f"wv{dt_}")
        nc.gpsimd.dma_start(w[:], wv_d[dt_ * 128:(dt_ + 1) * 128, :])
        wtv.append(w)

    for c in range(1, NCH):
        for dt_ in range(NDT):
            nc.sync.dma_start(
                xT[dt_][c][:],
                xT_d[dt_ * 128:(dt_ + 1) * 128, c * 512:(c + 1) * 512])

    bqcol, bkcol = [], []
    for pair in range(4):
        bc_ = bpool.tile([128, 1], f32, tag=f"bq{pair}", name=f"bq{pair}")
        nc.gpsimd.dma_start(bc_[:], bq_d[pair * 128:(pair + 1) * 128, None])
        bqcol.append(bc_)
        bc2 = bpool.tile([128, 1], f32, tag=f"bk{pair}", name=f"bk{pair}")
        nc.gpsimd.dma_start(bc2[:], bk_d[pair * 128:(pair + 1) * 128, None])
        bkcol.append(bc2)
    bvrow = bpool.tile([1, F], f32, tag="bvrow")
    nc.gpsimd.dma_start(bvrow[:], bv_d[None, :])
    bvb = bpool.tile([128, F], f32, tag="bvb")
    nc.gpsimd.partition_broadcast(bvb[:], bvrow[:])

    wp = []
    for ft in range(4):
        w = wpool.tile([128, D], bf16, tag=f"wp{ft}", name=f"wp{ft}")
        nc.gpsimd.dma_start(w[:], wp_d[ft * 128:(ft + 1) * 128, :])
        wp.append(w)

    # ---------------- persistent SBUF arrays -----------------
    qT = [[qkpool.tile([128, 512], bf16, tag=f"qT{p}_{c}", name=f"qT{p}_{c}")
           for c in range(NCH)] for p in range(4)]
    kT = [[qkpool.tile([128, 512], bf16, tag=f"kT{p}_{c}", name=f"kT{p}_{c}")
           for c in range(NCH)] for p in range(4)]
    # v tiles: [128 keys, 8 heads, 64+1] -- col 64 of each head is 1.0 so the
    # AV matmul row 64 accumulates the softmax denominator.
    v_sb = [vpool.tile([128, NHL, HD + 1], bf16, tag=f"v{st}", name=f"v{st}")
            for st in range(NST)]
    aoT = [[aopool.tile([128, 512], bf16, tag=f"ao{p}_{c}", name=f"ao{p}_{c}")
            for c in range(NCH)] for p in range(4)]

    def qk_half(pair, cs, wsb, bcol, dst):
        # both chunks stream through the same stationary weight tile per dt
        # so the PE can reuse the loaded weights (halves LDWEIGHTS traffic)
        pss = [ps_pr.tile([128, 512], f32, tag="pr", name=f"qk{pair}_{c}")
               for c in cs]
        for dt_ in range(NDT):
            for ps, c in zip(pss, cs):
                nc.tensor.matmul(
                    ps[:], wsb[dt_][:, pair * 128:(pair + 1) * 128],
                    xT[dt_][c][:],
                    start=(dt_ == 0), stop=(dt_ == NDT - 1))
            if dt_ < NDT - 1:
                yield
        for ps, c in zip(pss, cs):
            nc.vector.tensor_scalar_add(dst[pair][c][:], ps[:], bcol[pair][:])
        yield

    def gen_qk(pair, cs):
        yield from qk_half(pair, cs, wq_sb, bqcol, qT)
        yield from qk_half(pair, cs, wk_sb, bkcol, kT)

    def gen_v(st):
        ps = ps_pr.tile([128, 512], f32, tag="pr", name=f"v{st}")
        for dt_ in range(NDT):
            nc.tensor.matmul(
                ps[:], xT[dt_][st // 4][:, (st % 4) * 128:(st % 4 + 1) * 128],
                wtv[dt_][:],
                start=(dt_ == 0), stop=(dt_ == NDT - 1))
            if dt_ % 2 == 1 and dt_ < NDT - 1:
                yield
        nc.vector.tensor_add(
            v_sb[st][:, :, 0:HD],
            ps[:].rearrange("p (h d) -> p h d", h=NHL),
            bvb[:].rearrange("p (h d) -> p h d", h=NHL))
        nc.vector.memset(v_sb[st][:, :, HD:HD + 1], 1.0)
        yield

    def gen_proj(c, st):
        # both d-halves share each stationary aoT slice (LDWEIGHTS reuse)
        pos = [ps_pr.tile([128, 512], f32, tag="pr", name=f"po{st}_{n}")
               for n in range(2)]
        for p4 in range(4):
            for nch_ in range(2):
                nc.tensor.matmul(
                    pos[nch_][:],
                    aoT[p4][c][:, (st % 4) * 128:(st % 4 + 1) * 128],
                    wp[p4][:, nch_ * 512:(nch_ + 1) * 512],
                    start=(p4 == 0), stop=(p4 == 3))
            if p4 < 3:
                yield
        for nch_ in range(2):
            ot = ostage.tile([128, 512], f32, tag="ot", name=f"ot{st}_{nch_}")
            nc.vector.tensor_copy(ot[:], pos[nch_][:])
            nc.sync.dma_start(
                out_d[st * 128:(st + 1) * 128,
                      nch_ * 512:(nch_ + 1) * 512], ot[:])
        yield

    # ---- filler scheduling: the per-engine queues execute in program
    # order, so PE filler work (qkv / output projections) must be emitted
    # BETWEEN attention matmuls in ~2-matmul quanta — big lumps would delay
    # the next scores matmul and starve ACT.
    filler_q = []           # list of (tags_tuple, generator_fn)
    emitted = set()
    cur = [None, None]      # (tags_tuple, running generator)

    def pump(steps):
        for _ in range(steps):
            if cur[1] is None:
                if not filler_q:
                    return
                tags, genf = filler_q.pop(0)
                cur[0], cur[1] = tags, genf()
            try:
                next(cur[1])
            except StopIteration:
                emitted.update(cur[0])
                cur[1] = None

    def require(tags):
        for _ in range(10000):
            if all(t in emitted for t in tags):
                return
            assert filler_q or cur[1] is not None, f"missing {tags}"
            pump(1)
        raise AssertionError("require did not converge")

    def norm_chunk(pair, c, pav):
        # normalize: aoT[pair][c] = pav * (1/denom) broadcast on partitions
        for i in range(2):
            sums_sb = npool.tile([1, 512], f32, tag="sums",
                                 name=f"s{pair}_{c}_{i}")
            nc.vector.tensor_copy(sums_sb[:], pav[i][HD:HD + 1, :])
            r = npool.tile([1, 512], f32, tag="r", name=f"r{pair}_{c}_{i}")
            nc.vector.reciprocal_approx_fast(r[:], sums_sb[:])
            bc_ = npool.tile([HD, 512], f32, tag="bc",
                             name=f"bc{pair}_{c}_{i}")
            nc.gpsimd.partition_broadcast(bc_[:], r[:])
            nc.vector.tensor_mul(aoT[pair][c][64 * i:64 * i + 64, :],
                                 pav[i][0:HD, :], bc_[:])

    # ---------------- emission order -----------------
    for _ in gen_qk(0, (0,)):
        pass
    emitted.add("qk0_0")
    for st in range(0, 4):
        filler_q.append(((f"v{st}",), lambda st=st: gen_v(st)))
    filler_q.append((("qk0_1",), lambda: gen_qk(0, (1,))))
    for st in range(4, 8):
        filler_q.append(((f"v{st}",), lambda st=st: gen_v(st)))
    filler_q.append((("qk0_2", "qk0_3"), lambda: gen_qk(0, (2, 3))))
    for st in range(8, 16):
        filler_q.append(((f"v{st}",), lambda st=st: gen_v(st)))
    for p in range(1, 4):
        for cp in range(2):
            filler_q.append(((f"qk{p}_{2 * cp}", f"qk{p}_{2 * cp + 1}"),
                             lambda p=p, cp=cp: gen_qk(p, (2 * cp,
                                                          2 * cp + 1))))

    # Flat software-pipelined attention stream over all (pair, chunk, kt):
    # AV matmuls trail their exp by one global step, across chunk and pair
    # boundaries, so the PE queue never knots on an ACT-dependent matmul.
    steps = [(pair, c, kt) for pair in range(4) for c in range(NCH)
             for kt in range(4 * c + 4)]
    pav_cur = {}
    pending = None      # (pair, c, kt, pp, lo, is_last)

    def flush_pending():
        if pending is None:
            return
        pair, c, kt, pp, lo, is_last = pending
        require([f"v{kt}"])
        if kt == 0:
            pav_cur[(pair, c)] = [
                ps_av.tile([HD + 1, 512], f32, tag="av",
                           name=f"av{pair}_{c}_{i}") for i in range(2)]
        pav = pav_cur[(pair, c)]
        for i in range(2):
            nc.tensor.matmul(pav[i][:, lo:512],
                             v_sb[kt][:, 2 * pair + i, :],
                             pp[:, i, lo:512],
                             start=(kt == 0), stop=is_last)
        if is_last:
            norm_chunk(pair, c, pav)
            del pav_cur[(pair, c)]
            if pair == 3:
                for st in range(4 * c, 4 * c + 4):
                    filler_q.append(
                        ((f"po{st}",),
                         lambda c=c, st=st: gen_proj(c, st)))

    for pair, c, kt in steps:
        if kt == 0:
            require([f"qk{pair}_{c}"])
        j = kt - 4 * c          # >= 0 on diagonal k-tiles
        lo = 0 if j < 0 else 128 * j
        sc = ps_sc.tile([128, 2, 512], f32, tag="sc")
        for i in range(2):
            nc.tensor.matmul(
                sc[:, i, lo:512],
                kT[pair][kt // 4][64 * i:64 * i + 64,
                                  (kt % 4) * 128:(kt % 4 + 1) * 128],
                qT[pair][c][64 * i:64 * i + 64, lo:512],
                start=True, stop=True)
        pp = ppool.tile([128, 2, 512], bf16, tag="pp")
        nc.scalar.activation(pp[:, :, lo:512], sc[:, :, lo:512], Exp,
                             scale=SCALE)
        if j >= 0:
            nc.vector.tensor_mul(
                pp[:, :, lo:lo + 128], pp[:, :, lo:lo + 128],
                tri[:, None, :].broadcast_to([128, 2, 128]))
        pump(2 if pair == 0 else 1)
        flush_pending()
        pending = (pair, c, kt, pp, lo, kt == 4 * c + 3)
    flush_pending()
    pending = None
    while filler_q or cur[1] is not None:
        pump(1)

    for pool in (ostage, npool, ppool, ps_av, ps_sc, ps_pr, aopool,
                 vpool, qkpool, xpool, bpool, wpool, consts):
        pool.release()


_NC_CACHE = None


def _get_program():
    global _NC_CACHE
    if _NC_CACHE is None:
        _NC_CACHE = build_program()
    return _NC_CACHE


def make_in_maps(x, qkv_w, qkv_b, proj_w):
    in_maps = []
    for cid in range(N_CORES):
        b, hg = cid // 2, cid % 2
        bf = ml_dtypes.bfloat16
        in_maps.append({
            "xT": np.ascontiguousarray(x[b].T).astype(bf),
            "wq": np.ascontiguousarray(qkv_w[:, hg * F:(hg + 1) * F]).astype(bf),
            "wk": np.ascontiguousarray(qkv_w[:, D + hg * F:D + (hg + 1) * F]).astype(bf),
            "wv": np.ascontiguousarray(qkv_w[:, 2 * D + hg * F:2 * D + (hg + 1) * F]).astype(bf),
            "bq": np.ascontiguousarray(qkv_b[hg * F:(hg + 1) * F]),
            "bk": np.ascontiguousarray(qkv_b[D + hg * F:D + (hg + 1) * F]),
            "bv": np.ascontiguousarray(qkv_b[2 * D + hg * F:2 * D + (hg + 1) * F]),
            "wp": np.ascontiguousarray(proj_w[hg * F:(hg + 1) * F, :]).astype(bf),
        })
    return in_maps


LAST_RESULTS = None


def kernel(x, qkv_w, qkv_b, proj_w, proj_b, _trace=False):
    global LAST_RESULTS
    nc = _get_program()
    in_maps = make_in_maps(np.asarray(x, dtype=np.float32),
                           np.asarray(qkv_w, dtype=np.float32),
                           np.asarray(qkv_b, dtype=np.float32),
                           np.asarray(proj_w, dtype=np.float32))
    res = run_bass_kernel_spmd(nc, in_maps, core_ids=list(range(N_CORES)),
                               trace=_trace)
    LAST_RESULTS = res
    out = np.empty((B, S, D), dtype=np.float32)
    for b in range(B):
        out[b] = res.results[2 * b]["out"] + res.results[2 * b + 1]["out"]
    out += proj_b.astype(np.float32)
    return out
